# revision 13
# baseline (speedup 1.0000x reference)
"""Trainium2 Bass kernel for nn_MixedLoraModel_734.

Computes, for T=8192 tokens, D=4096:
    out = x @ W_base^T + b_base + scaling[token_lora][:,None] * lora(x)
where lora(x)[t] = WB[l_t] @ (WA[l_t] @ x[t]),  l_t = token_lora[t],
L=8 adapters of rank R=16 (the full adapter stack is 8*16 = 128 rows).

Strategy (8 NeuronCores, data-parallel over tokens):
  - Each core receives ONE bf16 blob holding its operands pre-laid-out
    host-side with the contraction dim on partitions:
      xT_sw   x shard transposed, partition-major swizzled
              xT_sw[p, c*TS + t] = x[t, c*128 + p]
      wT      [D, O] W_base transposed (natural [d, o] row-major)
      waT_sw  WA stack transposed, partition-major swizzled
      wbsT    [LR, O]  wbsT[16l+r, o] = scaling[l] * WB[l, o, r]
      mask    [LR, TS] mask[j, t] = (token_lora[t] == j // 16)
      bias    [O], ones [P]
    One packed tensor keeps the per-call PJRT dispatch cost down
    (2 buffer handles instead of 9).
  - Device-side the kernel is pure matmul streaming, no PE transposes:
      u[j, t]  = sum_d waT[d, j] * xT[d, t]       (dense, all adapters)
      u_m      = u * mask                          (per-token selection)
      acc[t,o] = sum_d xT[d, t] * wT[d, o]         (base GEMM, PSUM f32)
               + sum_j u_m[j, t] * wbsT[j, o]      (LoRA, same PSUM)
    eviction adds the (PE-broadcast, f32-resident) bias and DMAs out.
    All matmul operands are bf16 (full PE rate; PSUM accumulates f32;
    abs error ~1e-2 vs the checker's 0.11 tolerance at output scale).
  - DMA instruction count is minimized (HWDGE costs ~625ns per DMA):
    xT in 8 swizzled 8KB-line DMAs, W in 64 three-dim-AP DMAs of
    [128, 8 d-chunks, 512 o], outputs in 64 two-token-tile DMAs.
  - PSUM: 6 banks rotate as [128,512] accumulators (4 per 256-wide
    o-chunk), 2 banks for the u accumulation / bias broadcast.
"""

import numpy as np
import ml_dtypes

import concourse.bass as bass
import concourse.mybir as mybir
import concourse.tile as tile
from concourse import bacc

P = 128
D = 4096          # d_in
O = 4096          # d_out
NCORES = 8
T = 8192
TS = T // NCORES  # 1024 tokens per core
NT = TS // P      # 8 token tiles per core
ND = D // P       # 32 contraction chunks
OC = 256          # output-chunk width
NOC = O // OC     # 16
L, R, LR = 8, 16, 128

F32 = mybir.dt.float32
BF16 = mybir.dt.bfloat16
BF = ml_dtypes.bfloat16
MUL = mybir.AluOpType.mult
ADD = mybir.AluOpType.add

# blob layout (bf16 element offsets)
OFF_XT = 0                      # [P, ND*TS] swizzled
OFF_WT = OFF_XT + D * TS        # [D, O]
OFF_WAT = OFF_WT + D * O        # [P, ND*LR] swizzled
OFF_WBST = OFF_WAT + D * LR     # [LR, O]
OFF_MASK = OFF_WBST + LR * O    # [LR, TS]
OFF_BIAS = OFF_MASK + LR * TS   # [O]
OFF_ONES = OFF_BIAS + O         # [P]
N_BLOB = OFF_ONES + P


def _build() -> bass.Bass:
    nc = bacc.Bacc(None)

    blob = nc.declare_dram_parameter("blob", [N_BLOB], BF16, isOutput=False)
    out = nc.declare_dram_parameter("out", [TS, O], F32, isOutput=True)

    xT_d = blob[OFF_XT:OFF_XT + D * TS].rearrange("(a b) -> a b", b=ND * TS)
    wT_d = blob[OFF_WT:OFF_WT + D * O].rearrange("(a b) -> a b", b=D * O // P)
    waT_d = blob[OFF_WAT:OFF_WAT + D * LR].rearrange("(a b) -> a b", b=ND * LR)
    wbsT_d = blob[OFF_WBST:OFF_WBST + LR * O].rearrange("(a b) -> a b", b=O)
    mask_d = blob[OFF_MASK:OFF_MASK + LR * TS].rearrange("(a b) -> a b", b=TS)
    bias_d = blob[OFF_BIAS:OFF_BIAS + O].rearrange("(a b) -> a b", a=1)
    ones_d = blob[OFF_ONES:OFF_ONES + P].rearrange("(a b) -> a b", a=1)

    with tile.TileContext(nc) as tc:
        with (
            tc.tile_pool(name="res", bufs=1) as res,
            tc.tile_pool(name="wtp", bufs=8) as wtp,
            tc.tile_pool(name="outp", bufs=4) as outp,
            tc.tile_pool(name="acc_ps", bufs=6, space="PSUM") as acc_ps,
            tc.tile_pool(name="u_ps", bufs=2, space="PSUM") as u_ps,
        ):
            xTb = res.tile([P, ND * TS], BF16, tag="xTb")
            wbsT = res.tile([P, O], BF16, tag="wbsT")
            waT = res.tile([P, ND * LR], BF16, tag="waT")
            maskB = res.tile([P, TS], BF16, tag="maskB")
            maskF = res.tile([P, TS], F32, tag="maskF")
            u_mT = res.tile([P, TS], BF16, tag="u_mT")
            bias_row = res.tile([1, O], BF16, tag="bias_row")
            ones_col = res.tile([1, P], BF16, tag="ones")
            bias_sb = res.tile([P, O], F32, tag="bias_sb")

            # -------- input DMAs (order = queue order on the SP engine) ----
            nc.sync.dma_start(out=bias_row[:], in_=bias_d)
            nc.sync.dma_start(out=ones_col[:], in_=ones_d)

            # bias broadcast to all 128 partitions, resident f32 (fills the
            # head while the big DMAs stream)
            for bb in range(8):
                bps = u_ps.tile([P, 512], F32, tag="ups", name=f"bias_ps{bb}")
                nc.tensor.matmul(bps[:], ones_col[0:1, :],
                                 bias_row[0:1, bb * 512:(bb + 1) * 512],
                                 start=True, stop=True)
                nc.any.tensor_copy(bias_sb[:, bb * 512:(bb + 1) * 512], bps[:])

            def wt_fetch(ocp, dq):
                """One DMA: d-chunks dq*8..dq*8+7, o = ocp*512..ocp*512+512.

                W is pre-swizzled host-side so each (ocp, dq) tile is one
                contiguous 8KB line per partition (128 descriptors/DMA).
                """
                wtb = wtp.tile([P, 8 * 512], BF16, tag="wtb",
                               name=f"wtb{ocp}_{dq}")
                blk = (ocp * 4 + dq) * 4096
                nc.sync.dma_start(out=wtb[:], in_=wT_d[:, blk:blk + 4096])
                return wtb

            # xT eighths (1MB each, 8KB lines) interleaved with the first
            # o-pair's W so the PE can start at the first chunk
            wtb0 = []
            for q in range(8):
                nc.sync.dma_start(
                    out=xTb[:, q * 4 * TS:(q + 1) * 4 * TS],
                    in_=xT_d[:, q * 4 * TS:(q + 1) * 4 * TS])
                if q % 2 == 0:
                    wtb0.append(wt_fetch(0, q // 2))
                if q == 0:
                    nc.sync.dma_start(out=maskB[:], in_=mask_d)
                elif q == 2:
                    nc.sync.dma_start(out=waT[:], in_=waT_d)
            nc.sync.dma_start(out=wbsT[:], in_=wbsT_d)
            nc.vector.tensor_copy(maskF[:], maskB[:])

            ups = [u_ps.tile([P, 512], F32, tag="ups", name=f"ups{g}")
                   for g in range(2)]

            def emit_half(ocp, tg, wtbs, fuse_u=False, out_engines=None):
                """One 512-wide o-chunk for token tiles tg*4..tg*4+3.

                512-wide moving operands mean each PE stationary load is
                amortized over 512 cycles (the cost model ignores LdWeights
                but hardware does not), and each matmul fills exactly one
                PSUM bank.
                """
                o0 = ocp * 512
                accs = [acc_ps.tile([P, 512], F32, tag="acc",
                                    name=f"acc{ocp}_{tg}_{i}") for i in range(4)]
                for dc in range(ND):
                    rhs = wtbs[dc // 8][:, (dc % 8) * 512:(dc % 8) * 512 + 512]
                    for i in range(4):
                        tt = tg * 4 + i
                        nc.tensor.matmul(
                            accs[i][:],
                            xTb[:, dc * TS + tt * P:dc * TS + (tt + 1) * P],
                            rhs,
                            start=(dc == 0), stop=False)
                    if fuse_u:
                        # dense u for all adapters rides this dc sweep; the
                        # routing mask selects per-token rows afterwards
                        for g2 in range(2):
                            nc.tensor.matmul(
                                ups[g2][:],
                                waT[:, dc * LR:(dc + 1) * LR],
                                xTb[:, dc * TS + g2 * 512:dc * TS + g2 * 512 + 512],
                                start=(dc == 0), stop=(dc == ND - 1))
                if fuse_u:
                    for g2 in range(2):
                        nc.vector.tensor_tensor(
                            u_mT[:, g2 * 512:(g2 + 1) * 512], ups[g2][:],
                            maskF[:, g2 * 512:(g2 + 1) * 512], MUL)
                # LoRA accumulates into the same PSUM banks
                for i in range(4):
                    tt = tg * 4 + i
                    nc.tensor.matmul(
                        accs[i][:],
                        u_mT[:, tt * P:(tt + 1) * P],
                        wbsT[:, o0:o0 + 512],
                        start=False, stop=True)
                # evict with bias add; one two-token-tile DMA per osb
                for j in range(2):
                    osb = outp.tile([P, 1024], F32, tag="osb",
                                    name=f"osb{ocp}_{tg}_{j}")
                    for i in (2 * j, 2 * j + 1):
                        nc.any.tensor_tensor(
                            osb[:, (i % 2) * 512:(i % 2) * 512 + 512],
                            accs[i][:], bias_sb[:, o0:o0 + 512], ADD)
                    t0 = (tg * 4 + 2 * j) * P
                    dst = out[t0:t0 + 2 * P, o0:o0 + 512] \
                        .rearrange("(h p) o -> p h o", p=P)
                    src = osb[:].rearrange("p (h o) -> p h o", o=512)
                    eng = nc.scalar if out_engines is None else out_engines[j]
                    eng.dma_start(out=dst, in_=src)

            # first o-chunk's first token group carries the fused u sweep
            emit_half(0, 0, wtb0, fuse_u=True)
            emit_half(0, 1, wtb0)
            for ocp in range(1, 8):
                wtbs = [wt_fetch(ocp, dq) for dq in range(4)]
                emit_half(ocp, 0, wtbs)
                # last chunk: drain outputs over both DMA queues (the SP
                # queue has no W fetches left to block)
                last_engines = [nc.scalar, nc.sync] if ocp == 7 else None
                emit_half(ocp, 1, wtbs, out_engines=last_engines)

    nc.finalize()
    return nc


_NC = None


def _get_nc():
    global _NC
    if _NC is None:
        _NC = _build()
    return _NC


class _Runner:
    """Cached PJRT executable for the SPMD bass kernel.

    Mirrors concourse.bass2jax.run_bass_via_pjrt's multi-core path but
    keeps the jitted shard_map callable alive across invocations so
    repeated kernel() calls skip retrace/recompile.
    """

    def __init__(self):
        import jax
        import concourse.mybir as mybir_
        from concourse import bass2jax

        bass2jax.install_neuronx_cc_hook()
        self._bass2jax = bass2jax
        nc = _get_nc()
        self.nc = nc

        partition_name = (nc.partition_id_tensor.name
                          if nc.partition_id_tensor else None)
        in_names, out_names, out_avals = [], [], []
        for alloc in nc.m.functions[0].allocations:
            if not isinstance(alloc, mybir_.MemoryLocationSet):
                continue
            name = alloc.memorylocations[0].name
            if alloc.kind == "ExternalInput":
                if name != partition_name:
                    in_names.append(name)
            elif alloc.kind == "ExternalOutput":
                shape = tuple(alloc.tensor_shape)
                dtype = mybir_.dt.np(alloc.dtype)
                out_names.append(name)
                out_avals.append(jax.core.ShapedArray(shape, dtype))
        self.in_names = list(in_names)
        self.out_names = out_names
        self.out_avals = out_avals
        all_in_names = in_names + out_names
        if partition_name is not None:
            all_in_names.append(partition_name)

        from jax.experimental.shard_map import shard_map
        from jax.sharding import Mesh, NamedSharding, PartitionSpec

        devices = jax.devices()[:NCORES]
        assert len(devices) == NCORES, devices
        mesh = Mesh(np.asarray(devices), ("core",))
        self.mesh = mesh

        n_in = len(in_names)
        in_specs = (PartitionSpec("core"),) * (n_in + len(out_names))
        out_specs = (PartitionSpec("core"),) * len(out_names)
        self.out_sharding = NamedSharding(mesh, PartitionSpec("core"))
        self.in_shardings = [self.out_sharding] * n_in

        def _body(*args):
            operands = list(args)
            if partition_name is not None:
                operands.append(bass2jax.partition_id_tensor())
            outs = bass2jax._bass_exec_p.bind(
                *operands,
                out_avals=tuple(out_avals),
                in_names=tuple(all_in_names),
                out_names=tuple(out_names),
                lowering_input_output_aliases=(),
                sim_require_finite=True,
                sim_require_nnan=True,
                nc=nc,
            )
            return tuple(outs)

        self._fn = jax.jit(
            shard_map(_body, mesh=mesh, in_specs=in_specs,
                      out_specs=out_specs, check_rep=False),
            keep_unused=True)
        # resident zero operands for the NEFF's output-tensor inputs (the
        # kernel writes every output element, so contents don't matter and
        # the same device buffers are reused every call)
        self._scratch_dev = [
            jax.device_put(
                np.zeros((NCORES * a.shape[0], *a.shape[1:]), a.dtype),
                self.out_sharding)
            for a in out_avals
        ]

    def put_inputs(self, by_name):
        import jax
        return [jax.device_put(by_name[name], sharding)
                for name, sharding in zip(self.in_names, self.in_shardings)]

    def run_device(self, dev_args):
        """dev_args: device arrays in in_names order. Returns jax arrays."""
        return self._fn(*dev_args, *self._scratch_dev)

    def run(self, by_name):
        outs = self.run_device(self.put_inputs(by_name))
        host = [np.asarray(o) for o in outs]
        return {n: h for n, h in zip(self.out_names, host)}


_RUNNER = None


def _get_runner():
    global _RUNNER
    if _RUNNER is None:
        _RUNNER = _Runner()
    return _RUNNER


def _pack_blobs(x, W_base, b_base, WA, WB, scaling, token_lora):
    """Host-side packing into one bf16 blob per core, concatenated on
    axis 0 (global [NCORES * N_BLOB] for the core-sharded runner)."""
    x = np.asarray(x, np.float32)
    W = np.asarray(W_base, np.float32)
    b = np.asarray(b_base, np.float32)
    WA_ = np.asarray(WA, np.float32)
    WB_ = np.asarray(WB, np.float32)
    sc = np.asarray(scaling, np.float32)
    tl = np.asarray(token_lora, np.int32)

    # wT swizzled for the streaming tiles: element [p, ocp, dq, c, o] =
    # W^T[dq*1024 + c*128 + p, ocp*512 + o], so each (ocp, dq) tile is
    # one contiguous 8KB per-partition line
    wT = np.ascontiguousarray(
        W.T.reshape(4, 8, P, 8, 512).transpose(2, 3, 0, 1, 4)
    ).astype(BF).ravel()
    # waT swizzled: waT_sw[p, c*LR + j] = WA_flat[j, c*128 + p]
    waT = np.ascontiguousarray(
        WA_.reshape(LR, D).T.reshape(ND, P, LR).transpose(1, 0, 2)
        .reshape(P, ND * LR)).astype(BF).ravel()
    wbsT = np.ascontiguousarray(
        (WB_ * sc[:, None, None]).transpose(0, 2, 1).reshape(LR, O)
    ).astype(BF).ravel()
    bias = b.astype(BF)
    ones = np.ones(P, BF)
    jdiv = (np.arange(LR) // R).astype(np.int32)

    blobs = np.empty((NCORES, N_BLOB), BF)
    for c in range(NCORES):
        xs = x[c * TS:(c + 1) * TS]
        tls = tl[c * TS:(c + 1) * TS]
        row = blobs[c]
        # xT swizzled: xT_sw[p, c*TS + t] = x[t, c*128 + p]
        row[OFF_XT:OFF_XT + D * TS] = np.ascontiguousarray(
            xs.T.reshape(ND, P, TS).transpose(1, 0, 2).reshape(P, ND * TS)
        ).astype(BF).ravel()
        row[OFF_WT:OFF_WT + D * O] = wT
        row[OFF_WAT:OFF_WAT + D * LR] = waT
        row[OFF_WBST:OFF_WBST + LR * O] = wbsT
        row[OFF_MASK:OFF_MASK + LR * TS] = \
            (jdiv[:, None] == tls[None, :]).astype(BF).ravel()
        row[OFF_BIAS:OFF_BIAS + O] = bias
        row[OFF_ONES:OFF_ONES + P] = ones
    return blobs


def _global_inputs(x, W_base, b_base, WA, WB, scaling, token_lora):
    """Full-size (global) arrays keyed by DRAM-parameter name."""
    blobs = _pack_blobs(x, W_base, b_base, WA, WB, scaling, token_lora)
    return {"blob": blobs.reshape(NCORES * N_BLOB)}


def kernel(x, W_base, b_base, WA, WB, scaling, token_lora):
    by_name = _global_inputs(x, W_base, b_base, WA, WB, scaling, token_lora)
    try:
        res = _get_runner().run(by_name)
        return res["out"]
    except Exception:
        # robust fallback through the library SPMD path
        from concourse.bass_utils import run_bass_kernel_spmd

        nc = _get_nc()
        blob = by_name["blob"].reshape(NCORES, N_BLOB)
        in_maps = [{"blob": blob[c]} for c in range(NCORES)]
        res = run_bass_kernel_spmd(nc, in_maps, core_ids=list(range(NCORES)))
        return np.concatenate(
            [res.results[c]["out"] for c in range(NCORES)], axis=0)


# revision 14
# speedup vs baseline: 1.0174x; 1.0174x over previous
"""Trainium2 Bass kernel for nn_MixedLoraModel_734.

Computes, for T=8192 tokens, D=4096:
    out = x @ W_base^T + b_base + scaling[token_lora][:,None] * lora(x)
where lora(x)[t] = WB[l_t] @ (WA[l_t] @ x[t]),  l_t = token_lora[t],
L=8 adapters of rank R=16 (the full adapter stack is 8*16 = 128 rows).

Strategy (8 NeuronCores, data-parallel over tokens):
  - Each core receives ONE bf16 blob holding its operands pre-laid-out
    host-side with the contraction dim on partitions:
      xT_sw   x shard transposed, partition-major swizzled
              xT_sw[p, c*TS + t] = x[t, c*128 + p]
      wT      [D, O] W_base transposed (natural [d, o] row-major)
      waT_sw  WA stack transposed, partition-major swizzled
      wbsT    [LR, O]  wbsT[16l+r, o] = scaling[l] * WB[l, o, r]
      mask    [LR, TS] mask[j, t] = (token_lora[t] == j // 16)
      bias    [O], ones [P]
    One packed tensor keeps the per-call PJRT dispatch cost down
    (2 buffer handles instead of 9).
  - Device-side the kernel is pure matmul streaming, no PE transposes:
      u[j, t]  = sum_d waT[d, j] * xT[d, t]       (dense, all adapters)
      u_m      = u * mask                          (per-token selection)
      acc[t,o] = sum_d xT[d, t] * wT[d, o]         (base GEMM, PSUM f32)
               + sum_j u_m[j, t] * wbsT[j, o]      (LoRA, same PSUM)
    eviction adds the (PE-broadcast, f32-resident) bias and DMAs out.
    All matmul operands are bf16 (full PE rate; PSUM accumulates f32;
    abs error ~1e-2 vs the checker's 0.11 tolerance at output scale).
  - DMA instruction count is minimized (HWDGE costs ~625ns per DMA):
    xT in 8 swizzled 8KB-line DMAs, W in 64 three-dim-AP DMAs of
    [128, 8 d-chunks, 512 o], outputs in 64 two-token-tile DMAs.
  - PSUM: 6 banks rotate as [128,512] accumulators (4 per 256-wide
    o-chunk), 2 banks for the u accumulation / bias broadcast.
"""

import numpy as np
import ml_dtypes

import concourse.bass as bass
import concourse.mybir as mybir
import concourse.tile as tile
from concourse import bacc

P = 128
D = 4096          # d_in
O = 4096          # d_out
NCORES = 8
T = 8192
TS = T // NCORES  # 1024 tokens per core
NT = TS // P      # 8 token tiles per core
ND = D // P       # 32 contraction chunks
OC = 256          # output-chunk width
NOC = O // OC     # 16
L, R, LR = 8, 16, 128

F32 = mybir.dt.float32
BF16 = mybir.dt.bfloat16
BF = ml_dtypes.bfloat16
MUL = mybir.AluOpType.mult
ADD = mybir.AluOpType.add

# blob layout (bf16 element offsets)
OFF_XT = 0                      # [P, ND*TS] swizzled
OFF_WT = OFF_XT + D * TS        # [D, O]
OFF_WAT = OFF_WT + D * O        # [P, ND*LR] swizzled
OFF_WBST = OFF_WAT + D * LR     # [LR, O]
OFF_MASK = OFF_WBST + LR * O    # [LR, TS]
OFF_BIAS = OFF_MASK + LR * TS   # [O]
OFF_ONES = OFF_BIAS + O         # [P]
N_BLOB = OFF_ONES + P


def _build() -> bass.Bass:
    nc = bacc.Bacc(None, enable_partition_id=False)

    blob = nc.declare_dram_parameter("blob", [N_BLOB], BF16, isOutput=False)
    out = nc.declare_dram_parameter("out", [TS, O], F32, isOutput=True)

    xT_d = blob[OFF_XT:OFF_XT + D * TS].rearrange("(a b) -> a b", b=ND * TS)
    wT_d = blob[OFF_WT:OFF_WT + D * O].rearrange("(a b) -> a b", b=D * O // P)
    waT_d = blob[OFF_WAT:OFF_WAT + D * LR].rearrange("(a b) -> a b", b=ND * LR)
    wbsT_d = blob[OFF_WBST:OFF_WBST + LR * O].rearrange("(a b) -> a b", b=O)
    mask_d = blob[OFF_MASK:OFF_MASK + LR * TS].rearrange("(a b) -> a b", b=TS)
    bias_d = blob[OFF_BIAS:OFF_BIAS + O].rearrange("(a b) -> a b", a=1)
    ones_d = blob[OFF_ONES:OFF_ONES + P].rearrange("(a b) -> a b", a=1)

    with tile.TileContext(nc) as tc:
        with (
            tc.tile_pool(name="res", bufs=1) as res,
            tc.tile_pool(name="wtp", bufs=8) as wtp,
            tc.tile_pool(name="outp", bufs=4) as outp,
            tc.tile_pool(name="acc_ps", bufs=6, space="PSUM") as acc_ps,
            tc.tile_pool(name="u_ps", bufs=2, space="PSUM") as u_ps,
        ):
            xTb = res.tile([P, ND * TS], BF16, tag="xTb")
            wbsT = res.tile([P, O], BF16, tag="wbsT")
            waT = res.tile([P, ND * LR], BF16, tag="waT")
            maskB = res.tile([P, TS], BF16, tag="maskB")
            maskF = res.tile([P, TS], F32, tag="maskF")
            u_mT = res.tile([P, TS], BF16, tag="u_mT")
            bias_row = res.tile([1, O], BF16, tag="bias_row")
            ones_col = res.tile([1, P], BF16, tag="ones")
            bias_sb = res.tile([P, O], F32, tag="bias_sb")

            # -------- input DMAs (order = queue order on the SP engine) ----
            nc.sync.dma_start(out=bias_row[:], in_=bias_d)
            nc.sync.dma_start(out=ones_col[:], in_=ones_d)

            # bias broadcast to all 128 partitions, resident f32 (fills the
            # head while the big DMAs stream)
            for bb in range(8):
                bps = u_ps.tile([P, 512], F32, tag="ups", name=f"bias_ps{bb}")
                nc.tensor.matmul(bps[:], ones_col[0:1, :],
                                 bias_row[0:1, bb * 512:(bb + 1) * 512],
                                 start=True, stop=True)
                nc.any.tensor_copy(bias_sb[:, bb * 512:(bb + 1) * 512], bps[:])

            def wt_fetch(ocp, dq):
                """One DMA: d-chunks dq*8..dq*8+7, o = ocp*512..ocp*512+512.

                W is pre-swizzled host-side so each (ocp, dq) tile is one
                contiguous 8KB line per partition (128 descriptors/DMA).
                """
                wtb = wtp.tile([P, 8 * 512], BF16, tag="wtb",
                               name=f"wtb{ocp}_{dq}")
                blk = (ocp * 4 + dq) * 4096
                nc.sync.dma_start(out=wtb[:], in_=wT_d[:, blk:blk + 4096])
                return wtb

            # xT eighths (1MB each, 8KB lines) interleaved with the first
            # o-pair's W so the PE can start at the first chunk
            wtb0 = []
            for q in range(8):
                nc.sync.dma_start(
                    out=xTb[:, q * 4 * TS:(q + 1) * 4 * TS],
                    in_=xT_d[:, q * 4 * TS:(q + 1) * 4 * TS])
                if q % 2 == 0:
                    wtb0.append(wt_fetch(0, q // 2))
                if q == 0:
                    nc.sync.dma_start(out=maskB[:], in_=mask_d)
                elif q == 2:
                    nc.sync.dma_start(out=waT[:], in_=waT_d)
            nc.sync.dma_start(out=wbsT[:], in_=wbsT_d)
            nc.vector.tensor_copy(maskF[:], maskB[:])

            ups = [u_ps.tile([P, 512], F32, tag="ups", name=f"ups{g}")
                   for g in range(2)]

            def emit_half(ocp, tg, wtbs, fuse_u=False, out_engines=None):
                """One 512-wide o-chunk for token tiles tg*4..tg*4+3.

                512-wide moving operands mean each PE stationary load is
                amortized over 512 cycles (the cost model ignores LdWeights
                but hardware does not), and each matmul fills exactly one
                PSUM bank.
                """
                o0 = ocp * 512
                accs = [acc_ps.tile([P, 512], F32, tag="acc",
                                    name=f"acc{ocp}_{tg}_{i}") for i in range(4)]
                for dc in range(ND):
                    rhs = wtbs[dc // 8][:, (dc % 8) * 512:(dc % 8) * 512 + 512]
                    for i in range(4):
                        tt = tg * 4 + i
                        nc.tensor.matmul(
                            accs[i][:],
                            xTb[:, dc * TS + tt * P:dc * TS + (tt + 1) * P],
                            rhs,
                            start=(dc == 0), stop=False)
                    if fuse_u:
                        # dense u for all adapters rides this dc sweep; the
                        # routing mask selects per-token rows afterwards
                        for g2 in range(2):
                            nc.tensor.matmul(
                                ups[g2][:],
                                waT[:, dc * LR:(dc + 1) * LR],
                                xTb[:, dc * TS + g2 * 512:dc * TS + g2 * 512 + 512],
                                start=(dc == 0), stop=(dc == ND - 1))
                if fuse_u:
                    for g2 in range(2):
                        nc.vector.tensor_tensor(
                            u_mT[:, g2 * 512:(g2 + 1) * 512], ups[g2][:],
                            maskF[:, g2 * 512:(g2 + 1) * 512], MUL)
                # LoRA accumulates into the same PSUM banks
                for i in range(4):
                    tt = tg * 4 + i
                    nc.tensor.matmul(
                        accs[i][:],
                        u_mT[:, tt * P:(tt + 1) * P],
                        wbsT[:, o0:o0 + 512],
                        start=False, stop=True)
                # evict with bias add; one two-token-tile DMA per osb
                for j in range(2):
                    osb = outp.tile([P, 1024], F32, tag="osb",
                                    name=f"osb{ocp}_{tg}_{j}")
                    for i in (2 * j, 2 * j + 1):
                        nc.any.tensor_tensor(
                            osb[:, (i % 2) * 512:(i % 2) * 512 + 512],
                            accs[i][:], bias_sb[:, o0:o0 + 512], ADD)
                    t0 = (tg * 4 + 2 * j) * P
                    dst = out[t0:t0 + 2 * P, o0:o0 + 512] \
                        .rearrange("(h p) o -> p h o", p=P)
                    src = osb[:].rearrange("p (h o) -> p h o", o=512)
                    eng = nc.scalar if out_engines is None else out_engines[j]
                    eng.dma_start(out=dst, in_=src)

            # first o-chunk's first token group carries the fused u sweep
            emit_half(0, 0, wtb0, fuse_u=True)
            emit_half(0, 1, wtb0)
            for ocp in range(1, 8):
                wtbs = [wt_fetch(ocp, dq) for dq in range(4)]
                emit_half(ocp, 0, wtbs)
                # last chunk: drain outputs over both DMA queues (the SP
                # queue has no W fetches left to block)
                last_engines = [nc.scalar, nc.sync] if ocp == 7 else None
                emit_half(ocp, 1, wtbs, out_engines=last_engines)

    nc.finalize()
    return nc


_NC = None


def _get_nc():
    global _NC
    if _NC is None:
        _NC = _build()
    return _NC


class _Runner:
    """Cached PJRT executable for the SPMD bass kernel.

    Mirrors concourse.bass2jax.run_bass_via_pjrt's multi-core path but
    keeps the jitted shard_map callable alive across invocations so
    repeated kernel() calls skip retrace/recompile.
    """

    def __init__(self):
        import jax
        import concourse.mybir as mybir_
        from concourse import bass2jax

        bass2jax.install_neuronx_cc_hook()
        self._bass2jax = bass2jax
        nc = _get_nc()
        self.nc = nc

        partition_name = (nc.partition_id_tensor.name
                          if nc.partition_id_tensor else None)
        in_names, out_names, out_avals = [], [], []
        for alloc in nc.m.functions[0].allocations:
            if not isinstance(alloc, mybir_.MemoryLocationSet):
                continue
            name = alloc.memorylocations[0].name
            if alloc.kind == "ExternalInput":
                if name != partition_name:
                    in_names.append(name)
            elif alloc.kind == "ExternalOutput":
                shape = tuple(alloc.tensor_shape)
                dtype = mybir_.dt.np(alloc.dtype)
                out_names.append(name)
                out_avals.append(jax.core.ShapedArray(shape, dtype))
        self.in_names = list(in_names)
        self.out_names = out_names
        self.out_avals = out_avals
        all_in_names = in_names + out_names
        if partition_name is not None:
            all_in_names.append(partition_name)

        from jax.experimental.shard_map import shard_map
        from jax.sharding import Mesh, NamedSharding, PartitionSpec

        devices = jax.devices()[:NCORES]
        assert len(devices) == NCORES, devices
        mesh = Mesh(np.asarray(devices), ("core",))
        self.mesh = mesh

        n_in = len(in_names)
        in_specs = (PartitionSpec("core"),) * (n_in + len(out_names))
        out_specs = (PartitionSpec("core"),) * len(out_names)
        self.out_sharding = NamedSharding(mesh, PartitionSpec("core"))
        self.in_shardings = [self.out_sharding] * n_in

        def _body(*args):
            operands = list(args)
            if partition_name is not None:
                operands.append(bass2jax.partition_id_tensor())
            outs = bass2jax._bass_exec_p.bind(
                *operands,
                out_avals=tuple(out_avals),
                in_names=tuple(all_in_names),
                out_names=tuple(out_names),
                lowering_input_output_aliases=(),
                sim_require_finite=True,
                sim_require_nnan=True,
                nc=nc,
            )
            return tuple(outs)

        self._fn = jax.jit(
            shard_map(_body, mesh=mesh, in_specs=in_specs,
                      out_specs=out_specs, check_rep=False),
            keep_unused=True)
        # resident zero operands for the NEFF's output-tensor inputs (the
        # kernel writes every output element, so contents don't matter and
        # the same device buffers are reused every call)
        self._scratch_dev = [
            jax.device_put(
                np.zeros((NCORES * a.shape[0], *a.shape[1:]), a.dtype),
                self.out_sharding)
            for a in out_avals
        ]

    def put_inputs(self, by_name):
        import jax
        return [jax.device_put(by_name[name], sharding)
                for name, sharding in zip(self.in_names, self.in_shardings)]

    def run_device(self, dev_args):
        """dev_args: device arrays in in_names order. Returns jax arrays."""
        return self._fn(*dev_args, *self._scratch_dev)

    def run(self, by_name):
        outs = self.run_device(self.put_inputs(by_name))
        host = [np.asarray(o) for o in outs]
        return {n: h for n, h in zip(self.out_names, host)}


_RUNNER = None


def _get_runner():
    global _RUNNER
    if _RUNNER is None:
        _RUNNER = _Runner()
    return _RUNNER


def _pack_blobs(x, W_base, b_base, WA, WB, scaling, token_lora):
    """Host-side packing into one bf16 blob per core, concatenated on
    axis 0 (global [NCORES * N_BLOB] for the core-sharded runner)."""
    x = np.asarray(x, np.float32)
    W = np.asarray(W_base, np.float32)
    b = np.asarray(b_base, np.float32)
    WA_ = np.asarray(WA, np.float32)
    WB_ = np.asarray(WB, np.float32)
    sc = np.asarray(scaling, np.float32)
    tl = np.asarray(token_lora, np.int32)

    # wT swizzled for the streaming tiles: element [p, ocp, dq, c, o] =
    # W^T[dq*1024 + c*128 + p, ocp*512 + o], so each (ocp, dq) tile is
    # one contiguous 8KB per-partition line
    wT = np.ascontiguousarray(
        W.T.reshape(4, 8, P, 8, 512).transpose(2, 3, 0, 1, 4)
    ).astype(BF).ravel()
    # waT swizzled: waT_sw[p, c*LR + j] = WA_flat[j, c*128 + p]
    waT = np.ascontiguousarray(
        WA_.reshape(LR, D).T.reshape(ND, P, LR).transpose(1, 0, 2)
        .reshape(P, ND * LR)).astype(BF).ravel()
    wbsT = np.ascontiguousarray(
        (WB_ * sc[:, None, None]).transpose(0, 2, 1).reshape(LR, O)
    ).astype(BF).ravel()
    bias = b.astype(BF)
    ones = np.ones(P, BF)
    jdiv = (np.arange(LR) // R).astype(np.int32)

    blobs = np.empty((NCORES, N_BLOB), BF)
    for c in range(NCORES):
        xs = x[c * TS:(c + 1) * TS]
        tls = tl[c * TS:(c + 1) * TS]
        row = blobs[c]
        # xT swizzled: xT_sw[p, c*TS + t] = x[t, c*128 + p]
        row[OFF_XT:OFF_XT + D * TS] = np.ascontiguousarray(
            xs.T.reshape(ND, P, TS).transpose(1, 0, 2).reshape(P, ND * TS)
        ).astype(BF).ravel()
        row[OFF_WT:OFF_WT + D * O] = wT
        row[OFF_WAT:OFF_WAT + D * LR] = waT
        row[OFF_WBST:OFF_WBST + LR * O] = wbsT
        row[OFF_MASK:OFF_MASK + LR * TS] = \
            (jdiv[:, None] == tls[None, :]).astype(BF).ravel()
        row[OFF_BIAS:OFF_BIAS + O] = bias
        row[OFF_ONES:OFF_ONES + P] = ones
    return blobs


def _global_inputs(x, W_base, b_base, WA, WB, scaling, token_lora):
    """Full-size (global) arrays keyed by DRAM-parameter name."""
    blobs = _pack_blobs(x, W_base, b_base, WA, WB, scaling, token_lora)
    return {"blob": blobs.reshape(NCORES * N_BLOB)}


def kernel(x, W_base, b_base, WA, WB, scaling, token_lora):
    by_name = _global_inputs(x, W_base, b_base, WA, WB, scaling, token_lora)
    try:
        res = _get_runner().run(by_name)
        return res["out"]
    except Exception:
        # robust fallback through the library SPMD path
        from concourse.bass_utils import run_bass_kernel_spmd

        nc = _get_nc()
        blob = by_name["blob"].reshape(NCORES, N_BLOB)
        in_maps = [{"blob": blob[c]} for c in range(NCORES)]
        res = run_bass_kernel_spmd(nc, in_maps, core_ids=list(range(NCORES)))
        return np.concatenate(
            [res.results[c]["out"] for c in range(NCORES)], axis=0)


# revision 15
# speedup vs baseline: 1.0641x; 1.0459x over previous
"""Trainium2 Bass kernel for nn_MixedLoraModel_734.

Computes, for T=8192 tokens, D=4096:
    out = x @ W_base^T + b_base + scaling[token_lora][:,None] * lora(x)
where lora(x)[t] = WB[l_t] @ (WA[l_t] @ x[t]),  l_t = token_lora[t],
L=8 adapters of rank R=16 (the full adapter stack is 8*16 = 128 rows).

Strategy (8 NeuronCores, data-parallel over tokens):
  - Each core receives ONE bf16 blob holding its operands pre-laid-out
    host-side with the contraction dim on partitions:
      xT_sw   x shard transposed, partition-major swizzled
              xT_sw[p, c*TS + t] = x[t, c*128 + p]
      wT      [D, O] W_base transposed (natural [d, o] row-major)
      waT_sw  WA stack transposed, partition-major swizzled
      wbsT    [LR, O]  wbsT[16l+r, o] = scaling[l] * WB[l, o, r]
      mask    [LR, TS] mask[j, t] = (token_lora[t] == j // 16)
      bias    [O], ones [P]
    One packed tensor keeps the per-call PJRT dispatch cost down
    (2 buffer handles instead of 9).
  - Device-side the kernel is pure matmul streaming, no PE transposes:
      u[j, t]  = sum_d waT[d, j] * xT[d, t]       (dense, all adapters)
      u_m      = u * mask                          (per-token selection)
      acc[t,o] = sum_d xT[d, t] * wT[d, o]         (base GEMM, PSUM f32)
               + sum_j u_m[j, t] * wbsT[j, o]      (LoRA, same PSUM)
    eviction adds the (PE-broadcast, f32-resident) bias and DMAs out.
    All matmul operands are bf16 (full PE rate; PSUM accumulates f32;
    abs error ~1e-2 vs the checker's 0.11 tolerance at output scale).
  - DMA instruction count is minimized (HWDGE costs ~625ns per DMA):
    xT in 8 swizzled 8KB-line DMAs, W in 64 three-dim-AP DMAs of
    [128, 8 d-chunks, 512 o], outputs in 64 two-token-tile DMAs.
  - PSUM: 6 banks rotate as [128,512] accumulators (4 per 256-wide
    o-chunk), 2 banks for the u accumulation / bias broadcast.
"""

import numpy as np
import ml_dtypes

import concourse.bass as bass
import concourse.mybir as mybir
import concourse.tile as tile
from concourse import bacc

P = 128
D = 4096          # d_in
O = 4096          # d_out
NCORES = 8
T = 8192
TS = T // NCORES  # 1024 tokens per core
NT = TS // P      # 8 token tiles per core
ND = D // P       # 32 contraction chunks
OC = 256          # output-chunk width
NOC = O // OC     # 16
L, R, LR = 8, 16, 128

F32 = mybir.dt.float32
BF16 = mybir.dt.bfloat16
BF = ml_dtypes.bfloat16
MUL = mybir.AluOpType.mult
ADD = mybir.AluOpType.add

# blob layout (bf16 element offsets)
OFF_XT = 0                      # [P, ND*TS] swizzled
OFF_WT = OFF_XT + D * TS        # [D, O]
OFF_WAT = OFF_WT + D * O        # [P, ND*LR] swizzled
OFF_WBST = OFF_WAT + D * LR     # [LR, O]
OFF_MASK = OFF_WBST + LR * O    # [LR, TS]
OFF_BIAS = OFF_MASK + LR * TS   # [O]
OFF_ONES = OFF_BIAS + O         # [P]
N_BLOB = OFF_ONES + P


def _build() -> bass.Bass:
    nc = bacc.Bacc(None, enable_partition_id=False)

    blob = nc.declare_dram_parameter("blob", [N_BLOB], BF16, isOutput=False)
    out = nc.declare_dram_parameter("out", [TS, O], F32, isOutput=True)

    xT_d = blob[OFF_XT:OFF_XT + D * TS].rearrange("(a b) -> a b", b=ND * TS)
    wT_d = blob[OFF_WT:OFF_WT + D * O].rearrange("(a b) -> a b", b=D * O // P)
    waT_d = blob[OFF_WAT:OFF_WAT + D * LR].rearrange("(a b) -> a b", b=ND * LR)
    wbsT_d = blob[OFF_WBST:OFF_WBST + LR * O].rearrange("(a b) -> a b", b=O)
    mask_d = blob[OFF_MASK:OFF_MASK + LR * TS].rearrange("(a b) -> a b", b=TS)
    bias_d = blob[OFF_BIAS:OFF_BIAS + O].rearrange("(a b) -> a b", a=1)
    ones_d = blob[OFF_ONES:OFF_ONES + P].rearrange("(a b) -> a b", a=1)

    with tile.TileContext(nc) as tc:
        with (
            tc.tile_pool(name="res", bufs=1) as res,
            tc.tile_pool(name="wtp", bufs=8) as wtp,
            tc.tile_pool(name="outp", bufs=4) as outp,
            tc.tile_pool(name="acc_ps", bufs=6, space="PSUM") as acc_ps,
            tc.tile_pool(name="u_ps", bufs=2, space="PSUM") as u_ps,
        ):
            xTb = res.tile([P, ND * TS], BF16, tag="xTb")
            wbsT = res.tile([P, O], BF16, tag="wbsT")
            waT = res.tile([P, ND * LR], BF16, tag="waT")
            maskB = res.tile([P, TS], BF16, tag="maskB")
            maskF = res.tile([P, TS], F32, tag="maskF")
            u_mT = res.tile([P, TS], BF16, tag="u_mT")
            bias_row = res.tile([1, O], BF16, tag="bias_row")
            ones_col = res.tile([1, P], BF16, tag="ones")
            bias_sb = res.tile([P, O], F32, tag="bias_sb")

            # -------- input DMAs (order = queue order on the SP engine) ----
            nc.sync.dma_start(out=bias_row[:], in_=bias_d)
            nc.sync.dma_start(out=ones_col[:], in_=ones_d)

            # bias broadcast to all 128 partitions, resident f32 (fills the
            # head while the big DMAs stream)
            for bb in range(8):
                bps = u_ps.tile([P, 512], F32, tag="ups", name=f"bias_ps{bb}")
                nc.tensor.matmul(bps[:], ones_col[0:1, :],
                                 bias_row[0:1, bb * 512:(bb + 1) * 512],
                                 start=True, stop=True)
                nc.any.tensor_copy(bias_sb[:, bb * 512:(bb + 1) * 512], bps[:])

            def wt_fetch(ocp, dq):
                """One DMA: d-chunks dq*8..dq*8+7, o = ocp*512..ocp*512+512.

                W is pre-swizzled host-side so each (ocp, dq) tile is one
                contiguous 8KB line per partition (128 descriptors/DMA).
                """
                wtb = wtp.tile([P, 8 * 512], BF16, tag="wtb",
                               name=f"wtb{ocp}_{dq}")
                blk = (ocp * 4 + dq) * 4096
                nc.sync.dma_start(out=wtb[:], in_=wT_d[:, blk:blk + 4096])
                return wtb

            # xT eighths (1MB each, 8KB lines) interleaved with the first
            # o-pair's W so the PE can start at the first chunk
            wtb0 = []
            for q in range(8):
                nc.sync.dma_start(
                    out=xTb[:, q * 4 * TS:(q + 1) * 4 * TS],
                    in_=xT_d[:, q * 4 * TS:(q + 1) * 4 * TS])
                if q % 2 == 0:
                    wtb0.append(wt_fetch(0, q // 2))
                if q == 0:
                    nc.sync.dma_start(out=maskB[:], in_=mask_d)
                elif q == 2:
                    nc.sync.dma_start(out=waT[:], in_=waT_d)
            nc.sync.dma_start(out=wbsT[:], in_=wbsT_d)
            nc.vector.tensor_copy(maskF[:], maskB[:])

            ups = [u_ps.tile([P, 512], F32, tag="ups", name=f"ups{g}")
                   for g in range(2)]

            def emit_half(ocp, tg, wtbs, fuse_u=False, out_engines=None):
                """One 512-wide o-chunk for token tiles tg*4..tg*4+3.

                512-wide moving operands mean each PE stationary load is
                amortized over 512 cycles (the cost model ignores LdWeights
                but hardware does not), and each matmul fills exactly one
                PSUM bank.
                """
                o0 = ocp * 512
                accs = [acc_ps.tile([P, 512], F32, tag="acc",
                                    name=f"acc{ocp}_{tg}_{i}") for i in range(4)]
                for dc in range(ND):
                    rhs = wtbs[dc // 8][:, (dc % 8) * 512:(dc % 8) * 512 + 512]
                    for i in range(4):
                        tt = tg * 4 + i
                        nc.tensor.matmul(
                            accs[i][:],
                            xTb[:, dc * TS + tt * P:dc * TS + (tt + 1) * P],
                            rhs,
                            start=(dc == 0), stop=False)
                    if fuse_u:
                        # dense u for all adapters rides this dc sweep; the
                        # routing mask selects per-token rows afterwards
                        for g2 in range(2):
                            nc.tensor.matmul(
                                ups[g2][:],
                                waT[:, dc * LR:(dc + 1) * LR],
                                xTb[:, dc * TS + g2 * 512:dc * TS + g2 * 512 + 512],
                                start=(dc == 0), stop=(dc == ND - 1))
                if fuse_u:
                    for g2 in range(2):
                        nc.vector.tensor_tensor(
                            u_mT[:, g2 * 512:(g2 + 1) * 512], ups[g2][:],
                            maskF[:, g2 * 512:(g2 + 1) * 512], MUL)
                # LoRA accumulates into the same PSUM banks
                for i in range(4):
                    tt = tg * 4 + i
                    nc.tensor.matmul(
                        accs[i][:],
                        u_mT[:, tt * P:(tt + 1) * P],
                        wbsT[:, o0:o0 + 512],
                        start=False, stop=True)
                # evict with bias add; one two-token-tile DMA per osb
                for j in range(2):
                    osb = outp.tile([P, 1024], F32, tag="osb",
                                    name=f"osb{ocp}_{tg}_{j}")
                    for i in (2 * j, 2 * j + 1):
                        nc.any.tensor_tensor(
                            osb[:, (i % 2) * 512:(i % 2) * 512 + 512],
                            accs[i][:], bias_sb[:, o0:o0 + 512], ADD)
                    t0 = (tg * 4 + 2 * j) * P
                    dst = out[t0:t0 + 2 * P, o0:o0 + 512] \
                        .rearrange("(h p) o -> p h o", p=P)
                    src = osb[:].rearrange("p (h o) -> p h o", o=512)
                    eng = nc.scalar if out_engines is None else out_engines[j]
                    eng.dma_start(out=dst, in_=src)

            # first o-chunk's first token group carries the fused u sweep
            emit_half(0, 0, wtb0, fuse_u=True)
            emit_half(0, 1, wtb0)
            for ocp in range(1, 8):
                wtbs = [wt_fetch(ocp, dq) for dq in range(4)]
                emit_half(ocp, 0, wtbs)
                # last chunk: drain outputs over both DMA queues (the SP
                # queue has no W fetches left to block)
                last_engines = [nc.scalar, nc.sync] if ocp == 7 else None
                emit_half(ocp, 1, wtbs, out_engines=last_engines)

    nc.finalize()
    return nc


_NC = None


def _get_nc():
    global _NC
    if _NC is None:
        _NC = _build()
    return _NC


class _Runner:
    """Cached PJRT executable for the SPMD bass kernel.

    Mirrors concourse.bass2jax.run_bass_via_pjrt's multi-core path but
    keeps the jitted shard_map callable alive across invocations so
    repeated kernel() calls skip retrace/recompile.
    """

    def __init__(self):
        import jax
        import concourse.mybir as mybir_
        from concourse import bass2jax

        bass2jax.install_neuronx_cc_hook()
        self._bass2jax = bass2jax
        nc = _get_nc()
        self.nc = nc

        partition_name = (nc.partition_id_tensor.name
                          if nc.partition_id_tensor else None)
        in_names, out_names, out_avals = [], [], []
        for alloc in nc.m.functions[0].allocations:
            if not isinstance(alloc, mybir_.MemoryLocationSet):
                continue
            name = alloc.memorylocations[0].name
            if alloc.kind == "ExternalInput":
                if name != partition_name:
                    in_names.append(name)
            elif alloc.kind == "ExternalOutput":
                shape = tuple(alloc.tensor_shape)
                dtype = mybir_.dt.np(alloc.dtype)
                out_names.append(name)
                out_avals.append(jax.core.ShapedArray(shape, dtype))
        self.in_names = list(in_names)
        self.out_names = out_names
        self.out_avals = out_avals
        all_in_names = in_names + out_names
        if partition_name is not None:
            all_in_names.append(partition_name)

        from jax.experimental.shard_map import shard_map
        from jax.sharding import Mesh, NamedSharding, PartitionSpec

        devices = jax.devices()[:NCORES]
        assert len(devices) == NCORES, devices
        mesh = Mesh(np.asarray(devices), ("core",))
        self.mesh = mesh

        n_in = len(in_names)
        in_specs = (PartitionSpec("core"),) * (n_in + len(out_names))
        out_specs = (PartitionSpec("core"),) * len(out_names)
        self.out_sharding = NamedSharding(mesh, PartitionSpec("core"))
        self.in_shardings = [self.out_sharding] * n_in

        def _body(*args):
            operands = list(args)
            if partition_name is not None:
                operands.append(bass2jax.partition_id_tensor())
            outs = bass2jax._bass_exec_p.bind(
                *operands,
                out_avals=tuple(out_avals),
                in_names=tuple(all_in_names),
                out_names=tuple(out_names),
                lowering_input_output_aliases=(),
                sim_require_finite=True,
                sim_require_nnan=True,
                nc=nc,
            )
            return tuple(outs)

        self._fn = jax.jit(
            shard_map(_body, mesh=mesh, in_specs=in_specs,
                      out_specs=out_specs, check_rep=False),
            keep_unused=True)
        # resident zero operands for the NEFF's output-tensor inputs (the
        # kernel writes every output element, so contents don't matter and
        # the same device buffers are reused every call)
        self._scratch_dev = [
            jax.device_put(
                np.zeros((NCORES * a.shape[0], *a.shape[1:]), a.dtype),
                self.out_sharding)
            for a in out_avals
        ]

    def put_inputs(self, by_name):
        import jax
        return [jax.device_put(by_name[name], sharding)
                for name, sharding in zip(self.in_names, self.in_shardings)]

    def run_device(self, dev_args):
        """dev_args: device arrays in in_names order. Returns jax arrays."""
        return self._fn(*dev_args, *self._scratch_dev)

    def run(self, by_name):
        outs = self.run_device(self.put_inputs(by_name))
        host = [np.asarray(o) for o in outs]
        return {n: h for n, h in zip(self.out_names, host)}


_RUNNER = None


def _get_runner():
    global _RUNNER
    if _RUNNER is None:
        _RUNNER = _Runner()
    return _RUNNER


def _pack_blobs(x, W_base, b_base, WA, WB, scaling, token_lora):
    """Host-side packing into one bf16 blob per core, concatenated on
    axis 0 (global [NCORES * N_BLOB] for the core-sharded runner)."""
    x = np.asarray(x, np.float32)
    W = np.asarray(W_base, np.float32)
    b = np.asarray(b_base, np.float32)
    WA_ = np.asarray(WA, np.float32)
    WB_ = np.asarray(WB, np.float32)
    sc = np.asarray(scaling, np.float32)
    tl = np.asarray(token_lora, np.int32)

    # wT swizzled for the streaming tiles: element [p, ocp, dq, c, o] =
    # W^T[dq*1024 + c*128 + p, ocp*512 + o], so each (ocp, dq) tile is
    # one contiguous 8KB per-partition line
    wT = np.ascontiguousarray(
        W.T.reshape(4, 8, P, 8, 512).transpose(2, 3, 0, 1, 4)
    ).astype(BF).ravel()
    # waT swizzled: waT_sw[p, c*LR + j] = WA_flat[j, c*128 + p]
    waT = np.ascontiguousarray(
        WA_.reshape(LR, D).T.reshape(ND, P, LR).transpose(1, 0, 2)
        .reshape(P, ND * LR)).astype(BF).ravel()
    wbsT = np.ascontiguousarray(
        (WB_ * sc[:, None, None]).transpose(0, 2, 1).reshape(LR, O)
    ).astype(BF).ravel()
    bias = b.astype(BF)
    ones = np.ones(P, BF)
    jdiv = (np.arange(LR) // R).astype(np.int32)

    blobs = np.empty((NCORES, N_BLOB), BF)
    for c in range(NCORES):
        xs = x[c * TS:(c + 1) * TS]
        tls = tl[c * TS:(c + 1) * TS]
        row = blobs[c]
        # xT swizzled: xT_sw[p, c*TS + t] = x[t, c*128 + p]
        row[OFF_XT:OFF_XT + D * TS] = np.ascontiguousarray(
            xs.T.reshape(ND, P, TS).transpose(1, 0, 2).reshape(P, ND * TS)
        ).astype(BF).ravel()
        row[OFF_WT:OFF_WT + D * O] = wT
        row[OFF_WAT:OFF_WAT + D * LR] = waT
        row[OFF_WBST:OFF_WBST + LR * O] = wbsT
        row[OFF_MASK:OFF_MASK + LR * TS] = \
            (jdiv[:, None] == tls[None, :]).astype(BF).ravel()
        row[OFF_BIAS:OFF_BIAS + O] = bias
        row[OFF_ONES:OFF_ONES + P] = ones
    return blobs


def _global_inputs(x, W_base, b_base, WA, WB, scaling, token_lora):
    """Full-size (global) arrays keyed by DRAM-parameter name."""
    blobs = _pack_blobs(x, W_base, b_base, WA, WB, scaling, token_lora)
    return {"blob": blobs.reshape(NCORES * N_BLOB)}


def kernel(x, W_base, b_base, WA, WB, scaling, token_lora):
    import time

    by_name = _global_inputs(x, W_base, b_base, WA, WB, scaling, token_lora)
    try:
        try:
            res = _get_runner().run(by_name)
        except Exception:
            # a wedged NeuronCore (NRT_EXEC_UNIT_UNRECOVERABLE) from an
            # earlier process usually recovers on a later attempt
            time.sleep(10)
            res = _get_runner().run(by_name)
        return res["out"]
    except Exception:
        # robust fallback through the library SPMD path
        from concourse.bass_utils import run_bass_kernel_spmd

        nc = _get_nc()
        blob = by_name["blob"].reshape(NCORES, N_BLOB)
        in_maps = [{"blob": blob[c]} for c in range(NCORES)]
        res = run_bass_kernel_spmd(nc, in_maps, core_ids=list(range(NCORES)))
        return np.concatenate(
            [res.results[c]["out"] for c in range(NCORES)], axis=0)
